# revision 8
# baseline (speedup 1.0000x reference)
"""Multi-head causal self-attention forward on 8 Trainium2 NeuronCores.

Problem: x[4,2048,1024] @ w_qkv[1024,3072] -> causal MHA (16 heads, d=64)
         -> @ w_out[1024,1024] + b_out.

Sharding: core c handles batch b = c//2 and head-group g = c%2 (8 heads).
Each core computes a partial output  attn_out_heads(g) @ w_out[rows(g)]
for its batch; host sums the two partials per batch (row-parallel out
projection) and adds b_out.

Per-core kernel (bf16 matmul inputs, fp32 PSUM accumulate), organized to
keep the PE array continuously fed (TRN2 HAM drops the PE to half clock
after idle gaps):
  - q-group-outer / head-pair-inner loop. Attention for (qg, m) runs
    ST -> exp -> PV with sequentially processed head halves; all
    projection work (V, Q/K for later pairs, out-projection of finished
    q-groups) is chopped into ~1.7us "filler" units and injected
    between attention steps so the PE never idles while the ScalarE
    exp stream catches up.
  - attention in [k,q] orientation: S^T = kT.T @ qT per (k-chunk 128,
    q-group 1024); exp on ScalarE with scale=1/8 folded in; causal via
    block skip + triangular-mask multiply on the diagonal block;
    PV: u[65,512] += [v|1].T @ P accumulated over k-chunks ([v|1]
    carries a ones-column so u row 64 is the softmax denominator).
  - normalize with reciprocal_approx_fast + gpsimd partition_broadcast
    + DVE multiply -> hd tiles [hd, q] bf16.
  - out = hd.T @ w_out_shard accumulated over 4 hd chunks -> y f32,
    PSUM->SBUF copy on DVE (not ScalarE), DMA per [128,512] block.
  - input DMA split by token halves and ordered by first use (x+wq/wk
    first) so the first matmuls start ~5us in instead of waiting for
    the whole 8MB input set.
"""

import sys

sys.path.insert(0, "/opt/trn_rl_repo")

import numpy as np
import ml_dtypes

import concourse.bass as bass
import concourse.tile as tile
from concourse import bacc, mybir
from concourse.bass_utils import run_bass_kernel_spmd

BF16 = mybir.dt.bfloat16
F32 = mybir.dt.float32
NP_BF16 = ml_dtypes.bfloat16
EXP = mybir.ActivationFunctionType.Exp

B, T, C = 4, 2048, 1024
NCORES = 8
HC = 8  # heads per core
D = 64
DQ = HC * D  # 512
CA = C // 128  # 8 contraction chunks
NT128 = T // 128  # 16
SCALE = 1.0 / 8.0
LAG = 2  # ST/exp stream runs this many ik steps ahead of PV

_cached = None


def _build():
    nc = bacc.Bacc("TRN2", target_bir_lowering=False, debug=False, num_devices=NCORES)

    xT = nc.dram_tensor("xT", [C, T], BF16, kind="ExternalInput")
    wq = nc.dram_tensor("wq", [C, DQ], BF16, kind="ExternalInput")
    wk = nc.dram_tensor("wk", [C, DQ], BF16, kind="ExternalInput")
    wv = nc.dram_tensor("wv", [C, DQ], BF16, kind="ExternalInput")
    wo = nc.dram_tensor("wo", [DQ, C], BF16, kind="ExternalInput")
    trid = nc.dram_tensor("tri", [128, 128], BF16, kind="ExternalInput")
    y = nc.dram_tensor("y", [T, C], F32, kind="ExternalOutput")

    with tile.TileContext(nc) as tc:
        _emit(tc, nc, xT, wq, wk, wv, wo, trid, y)
    nc.compile()
    return nc


def _emit(tc, nc, xT, wq, wk, wv, wo, trid, y):
    from contextlib import ExitStack

    with ExitStack() as ctx:
        ep = ctx.enter_context

        persist = ep(tc.tile_pool(name="persist", bufs=1))
        qts = [persist.tile([128, T], BF16, tag=f"qt{m}", name=f"qt{m}") for m in range(4)]
        kts = [persist.tile([128, T], BF16, tag=f"kt{m}", name=f"kt{m}") for m in range(4)]
        vts = [persist.tile([128, HC * 65], BF16, tag=f"v{i}", name=f"v{i}") for i in range(NT128)]
        hds = [persist.tile([128, T], BF16, tag=f"hd{j}", name=f"hd{j}") for j in range(4)]
        wo_sb = [persist.tile([128, C], BF16, tag=f"wo{j}", name=f"wo{j}") for j in range(4)]
        tri = persist.tile([128, 128], BF16, tag="tri", name="tri")

        xin = ep(tc.tile_pool(name="xin", bufs=1))
        xts = [xin.tile([128, T], BF16, tag=f"x{a}", name=f"x{a}") for a in range(CA)]
        wq_sb = [xin.tile([128, DQ], BF16, tag=f"wq{a}", name=f"wqs{a}") for a in range(CA)]
        wk_sb = [xin.tile([128, DQ], BF16, tag=f"wk{a}", name=f"wks{a}") for a in range(CA)]
        wv_sb = [xin.tile([128, DQ], BF16, tag=f"wv{a}", name=f"wvs{a}") for a in range(CA)]

        # PSUM budget (8 banks): ST s_ps 2x[128,1024] = 4 banks,
        # u accumulators 2x[65,512] = 2 banks (halves run sequentially so
        # only one half's pair is live), shared filler pool 2x[128,512]
        # = 2 banks for V / QK projection / out-projection partials.
        stp = ep(tc.tile_pool(name="stp", bufs=2, space="PSUM"))
        u_ps = ep(tc.tile_pool(name="u_ps", bufs=2, space="PSUM"))
        fill_ps = ep(tc.tile_pool(name="fill_ps", bufs=2, space="PSUM"))
        p_pool = ep(tc.tile_pool(name="p_pool", bufs=12))
        norm = ep(tc.tile_pool(name="norm", bufs=4))
        ocp = ep(tc.tile_pool(name="ocp", bufs=2))

        # ---- input DMA, ordered by first use ----
        # Each transfer is worked by a single SDMA engine (~20 GB/s), with
        # ~16 in flight; completion time ~ transfer size. Quarter-split x so
        # the first-needed pieces land early instead of everything at ~27us.
        nc.sync.dma_start(out=tri, in_=trid[:, :])
        for a in range(CA):
            sl = slice(a * 128, (a + 1) * 128)
            nc.sync.dma_start(out=xts[a][:, 0:512], in_=xT[sl, 0:512])
            nc.sync.dma_start(out=wq_sb[a], in_=wq[sl, :])
            nc.sync.dma_start(out=wk_sb[a], in_=wk[sl, :])
        for a in range(CA):
            sl = slice(a * 128, (a + 1) * 128)
            nc.sync.dma_start(out=xts[a][:, 512:1024], in_=xT[sl, 512:1024])
        for a in range(CA):
            sl = slice(a * 128, (a + 1) * 128)
            nc.sync.dma_start(out=wv_sb[a], in_=wv[sl, :])
        for q in (2, 3):
            for a in range(CA):
                sl = slice(a * 128, (a + 1) * 128)
                nc.sync.dma_start(
                    out=xts[a][:, q * 512 : (q + 1) * 512], in_=xT[sl, q * 512 : (q + 1) * 512]
                )
        for j in range(4):
            nc.sync.dma_start(out=wo_sb[j], in_=wo[j * 128 : (j + 1) * 128, :])

        # ---- filler units (each ~0.9-1.7us of PE work) ----
        def emit_v(tk):
            ps = fill_ps.tile([128, 512], F32, tag="fps", name="vps")
            for a in range(CA):
                nc.tensor.matmul(
                    ps,
                    xts[a][:, tk * 128 : (tk + 1) * 128],
                    wv_sb[a],
                    start=(a == 0),
                    stop=(a == CA - 1),
                )
            v_view = vts[tk].rearrange("p (h e) -> p h e", e=65)
            nc.vector.tensor_copy(
                v_view[:, :, 0:64], ps.rearrange("p (h e) -> p h e", e=64)
            )
            nc.vector.memset(v_view[:, :, 64:65], 1.0)

        def emit_qkt_unit(m, wsb, dst, tbp, hb):
            ps = fill_ps.tile([128, 512], F32, tag="fps", name="qkps")
            lo = tbp * 1024 + hb * 512
            for a in range(CA):
                nc.tensor.matmul(
                    ps,
                    wsb[a][:, m * 128 : (m + 1) * 128],
                    xts[a][:, lo : lo + 512],
                    start=(a == 0),
                    stop=(a == CA - 1),
                )
            nc.vector.tensor_copy(dst[m][:, lo : lo + 512], ps)

        def emit_outproj_unit(tq, nb):
            ps = fill_ps.tile([128, 512], F32, tag="fps", name="opps")
            for j in range(4):
                nc.tensor.matmul(
                    ps,
                    hds[j][:, tq * 128 : (tq + 1) * 128],
                    wo_sb[j][:, nb * 512 : (nb + 1) * 512],
                    start=(j == 0),
                    stop=(j == 3),
                )
            ob = ocp.tile([128, 512], F32, tag="ob", name="ob")
            nc.vector.tensor_copy(ob, ps)
            nc.sync.dma_start(
                out=y[tq * 128 : (tq + 1) * 128, nb * 512 : (nb + 1) * 512],
                in_=ob,
            )

        # ---- attention ----
        def emit_st_exp(m, qg, ik, half):
            qlo = 1024 * qg
            kc = slice(ik * 128, (ik + 1) * 128)
            c0 = max(0, 128 * ik - qlo)
            rq = slice(half * 64, half * 64 + 64)
            s_ps = stp.tile([128, 1024], F32, tag="stps", name="sps")
            p_t = p_pool.tile([128, 1024], BF16, tag="p", name="pt")
            for qb in (2 * qg, 2 * qg + 1):
                if 4 * qb + 3 < ik:
                    continue
                lo = max(qb * 512, qlo + c0)  # global q start
                n = (qb + 1) * 512 - lo
                nc.tensor.matmul(
                    s_ps[:, lo - qlo : lo - qlo + n],
                    kts[m][rq, kc],
                    qts[m][rq, lo : lo + n],
                    start=True,
                    stop=True,
                )
            nc.scalar.activation(p_t[:, c0:1024], s_ps[:, c0:1024], EXP, scale=SCALE)
            if 128 * ik >= qlo:  # diagonal block: multiplicative causal mask
                nc.vector.tensor_mul(p_t[:, c0 : c0 + 128], p_t[:, c0 : c0 + 128], tri)
            return p_t

        def emit_pv_norm(m, qg, ik, half, p_t, us):
            qlo = 1024 * qg
            c0 = max(0, 128 * ik - qlo)
            h = 2 * m + half
            rq = slice(half * 64, half * 64 + 64)
            v_lhsT = vts[ik][:, h * 65 : h * 65 + 65]
            for qb in (2 * qg, 2 * qg + 1):
                if 4 * qb + 3 < ik:
                    continue
                a0 = qb * 512 - qlo
                off = max(0, c0 - a0)  # clip masked cols
                nc.tensor.matmul(
                    us[qb][:, off:512],
                    v_lhsT,
                    p_t[:, a0 + off : a0 + 512],
                    start=(ik == 0),
                    stop=(ik == 4 * qb + 3),
                    skip_group_check=True,
                )
                if ik == 4 * qb + 3:
                    # u complete: normalize into hd tiles
                    u = us[qb]
                    rec_in = norm.tile([1, 512], F32, tag="ri", name="ri")
                    nc.vector.tensor_copy(rec_in, u[64:65, :])
                    rec = norm.tile([1, 512], F32, tag="rc", name="rc")
                    nc.vector.reciprocal_approx_fast(out=rec, in_=rec_in)
                    bc = norm.tile([64, 512], F32, tag="bc", name="bc")
                    nc.gpsimd.partition_broadcast(bc, rec)
                    nc.vector.tensor_mul(
                        hds[m][rq, qb * 512 : (qb + 1) * 512], u[0:64, :], bc
                    )

        def attn(qg, m, fillers, front=0, late_fillers=()):
            """ST/exp -> PV pipeline over ik for each half; filler units are
            injected between steps. Tile treats emission order as program
            order, so a consumer emitted before its producer reads stale
            data: the first `front` fillers are paced one-per-step (for
            units consumed by THIS window with a small step margin, e.g. V
            feeding PV), the rest uniformly. `late_fillers` are emitted
            right after half 1's ik==11 PV (for out-proj units gated on
            this window's second-to-last normalize)."""
            iks = list(range(8 * (qg + 1)))
            nsteps = 2 * (len(iks) + LAG)
            step = 0
            fill_done = 0
            for half in range(2):
                staged = {}
                us = {}
                for t in range(len(iks) + LAG):
                    if t < len(iks):
                        staged[iks[t]] = emit_st_exp(m, qg, iks[t], half)
                    if t == LAG:
                        for qb in (2 * qg, 2 * qg + 1):
                            us[qb] = u_ps.tile([65, 512], F32, tag="u", name=f"u{qb}")
                    if t >= LAG:
                        ik = iks[t - LAG]
                        emit_pv_norm(m, qg, ik, half, staged.pop(ik), us)
                        if half == 1 and ik == 11 and late_fillers:
                            for u in late_fillers:
                                u()
                            late_fillers = ()
                    step += 1
                    if fill_done < front:
                        want = min(front, step)
                    else:
                        rem = len(fillers) - front
                        want = front + (step * rem) // nsteps
                    while fill_done < min(want, len(fillers)):
                        fillers[fill_done]()
                        fill_done += 1
            while fill_done < len(fillers):
                fillers[fill_done]()
                fill_done += 1

        def qkt_units(m, tbp):
            out = []
            for wsb, dst in ((wq_sb, qts), (wk_sb, kts)):
                for hb in range(2):
                    out.append(
                        lambda m=m, wsb=wsb, dst=dst, tbp=tbp, hb=hb: emit_qkt_unit(
                            m, wsb, dst, tbp, hb
                        )
                    )
            return out

        def op_units(tqs):
            return [
                lambda tq=tq, nb=nb: emit_outproj_unit(tq, nb)
                for tq in tqs
                for nb in range(2)
            ]

        # ---- schedule ----
        # Pre-phase: QK for pair 0 and the first half of V, emitted
        # serially (their consumers start in the very first window).
        for u in qkt_units(0, 0):
            u()
        for tk in range(8):
            emit_v(tk)

        # qg1 runs pairs in REVERSE order so the out-projection of qg0
        # (available once qg0 completes) can fill the late windows; the
        # last window's shortfall rolls into the dense out-proj tail.
        # V(8..15) is front-loaded into (1,3) (its PV consumes tk=ik with
        # a comfortable emission margin: V(15) by step 8, PV(ik15) at
        # step 18).
        windows = [
            (0, 0, qkt_units(1, 0), 0, ()),
            (0, 1, qkt_units(2, 0), 0, ()),
            (0, 2, qkt_units(3, 0), 0, ()),
            (0, 3, qkt_units(3, 1), 0, ()),
            (
                1,
                3,
                [lambda tk=tk: emit_v(tk) for tk in range(8, 16)]
                + qkt_units(2, 1)
                + op_units(range(0, 2)),
                8,
                (),
            ),
            (1, 2, qkt_units(1, 1) + op_units(range(2, 4)), 0, ()),
            (1, 1, qkt_units(0, 1) + op_units(range(4, 6)), 0, ()),
            (1, 0, op_units(range(6, 8)), 0, op_units(range(8, 12))),
        ]
        for qg, m, fillers, front, late in windows:
            attn(qg, m, fillers, front=front, late_fillers=late)
        for u in op_units(range(12, 16)):
            u()


def _in_maps(x, w_qkv, w_out):
    maps = []
    for c in range(NCORES):
        b, g = c // 2, c % 2
        h0 = g * DQ
        maps.append(
            {
                "xT": np.ascontiguousarray(x[b].T).astype(NP_BF16),
                "wq": w_qkv[:, h0 : h0 + DQ].astype(NP_BF16),
                "wk": w_qkv[:, C + h0 : C + h0 + DQ].astype(NP_BF16),
                "wv": w_qkv[:, 2 * C + h0 : 2 * C + h0 + DQ].astype(NP_BF16),
                "wo": np.ascontiguousarray(w_out[h0 : h0 + DQ, :]).astype(NP_BF16),
                "tri": np.triu(np.ones((128, 128), dtype=np.float32)).astype(NP_BF16),
            }
        )
    return maps


def get_bass():
    global _cached
    if _cached is None:
        _cached = _build()
    return _cached


def run(x, w_qkv, w_out, b_out, **spmd_kwargs):
    nc = get_bass()
    res = run_bass_kernel_spmd(
        nc, _in_maps(x, w_qkv, w_out), core_ids=list(range(NCORES)), **spmd_kwargs
    )
    out = np.empty((B, T, C), dtype=np.float32)
    for b in range(B):
        out[b] = res.results[2 * b]["y"] + res.results[2 * b + 1]["y"]
    out += b_out.astype(np.float32)
    return out, res


def kernel(x, w_qkv, w_out, b_out):
    x = np.asarray(x)
    w_qkv = np.asarray(w_qkv)
    w_out = np.asarray(w_out)
    b_out = np.asarray(b_out)
    out, _ = run(x, w_qkv, w_out, b_out)
    return out


if __name__ == "__main__":
    import reference

    inputs = {k: np.asarray(v) for k, v in reference.setup_inputs().items()}
    out = kernel(**inputs)
    print(out.shape, out.dtype)


# revision 12
# speedup vs baseline: 1.0423x; 1.0423x over previous
"""Multi-head causal self-attention forward on 8 Trainium2 NeuronCores.

Problem: x[4,2048,1024] @ w_qkv[1024,3072] -> causal MHA (16 heads, d=64)
         -> @ w_out[1024,1024] + b_out.

Sharding: core c handles batch b = c//2 and head-group g = c%2 (8 heads).
Each core computes a partial output  attn_out_heads(g) @ w_out[rows(g)]
for its batch; host sums the two partials per batch (row-parallel out
projection) and adds b_out.

Per-core kernel (bf16 matmul inputs, fp32 PSUM accumulate), organized to
keep the PE array continuously fed (TRN2 HAM drops the PE to half clock
after idle gaps):
  - q-group-outer / head-pair-inner loop. Attention for (qg, m) runs
    ST -> exp -> PV with sequentially processed head halves; all
    projection work (V, Q/K for later pairs, out-projection of finished
    q-groups) is chopped into ~1.7us "filler" units and injected
    between attention steps so the PE never idles while the ScalarE
    exp stream catches up.
  - attention in [k,q] orientation: S^T = kT.T @ qT per (k-chunk 128,
    q-group 1024); exp on ScalarE with scale=1/8 folded in; causal via
    block skip + triangular-mask multiply on the diagonal block;
    PV: u[65,512] += [v|1].T @ P accumulated over k-chunks ([v|1]
    carries a ones-column so u row 64 is the softmax denominator).
  - normalize with reciprocal_approx_fast + gpsimd partition_broadcast
    + DVE multiply -> hd tiles [hd, q] bf16.
  - out = hd.T @ w_out_shard accumulated over 4 hd chunks -> y f32,
    PSUM->SBUF copy on DVE (not ScalarE), DMA per [128,512] block.
  - input DMA split by token halves and ordered by first use (x+wq/wk
    first) so the first matmuls start ~5us in instead of waiting for
    the whole 8MB input set.
"""

import sys

sys.path.insert(0, "/opt/trn_rl_repo")

import numpy as np
import ml_dtypes

import concourse.bass as bass
import concourse.tile as tile
from concourse import bacc, mybir
from concourse.bass_utils import run_bass_kernel_spmd

BF16 = mybir.dt.bfloat16
F32 = mybir.dt.float32
NP_BF16 = ml_dtypes.bfloat16
EXP = mybir.ActivationFunctionType.Exp

B, T, C = 4, 2048, 1024
NCORES = 8
HC = 8  # heads per core
D = 64
DQ = HC * D  # 512
CA = C // 128  # 8 contraction chunks
NT128 = T // 128  # 16
SCALE = 1.0 / 8.0
LAG = 2  # ST/exp stream runs this many ik steps ahead of PV

_cached = None


def _build():
    nc = bacc.Bacc("TRN2", target_bir_lowering=False, debug=False, num_devices=NCORES)

    xT = nc.dram_tensor("xT", [C, T], BF16, kind="ExternalInput")
    wq = nc.dram_tensor("wq", [C, DQ], BF16, kind="ExternalInput")
    wk = nc.dram_tensor("wk", [C, DQ], BF16, kind="ExternalInput")
    wv = nc.dram_tensor("wv", [C, DQ], BF16, kind="ExternalInput")
    wo = nc.dram_tensor("wo", [DQ, C], BF16, kind="ExternalInput")
    trid = nc.dram_tensor("tri", [128, 128], BF16, kind="ExternalInput")
    y = nc.dram_tensor("y", [T, C], F32, kind="ExternalOutput")

    with tile.TileContext(nc) as tc:
        _emit(tc, nc, xT, wq, wk, wv, wo, trid, y)
    nc.compile()
    return nc


def _emit(tc, nc, xT, wq, wk, wv, wo, trid, y):
    from contextlib import ExitStack

    with ExitStack() as ctx:
        ep = ctx.enter_context

        persist = ep(tc.tile_pool(name="persist", bufs=1))
        qts = [persist.tile([128, T], BF16, tag=f"qt{m}", name=f"qt{m}") for m in range(4)]
        kts = [persist.tile([128, T], BF16, tag=f"kt{m}", name=f"kt{m}") for m in range(4)]
        vts = [persist.tile([128, HC * 65], BF16, tag=f"v{i}", name=f"v{i}") for i in range(NT128)]
        hds = [persist.tile([128, T], BF16, tag=f"hd{j}", name=f"hd{j}") for j in range(4)]
        wo_sb = [persist.tile([128, C], BF16, tag=f"wo{j}", name=f"wo{j}") for j in range(4)]
        tri = persist.tile([128, 128], BF16, tag="tri", name="tri")

        xin = ep(tc.tile_pool(name="xin", bufs=1))
        xts = [xin.tile([128, T], BF16, tag=f"x{a}", name=f"x{a}") for a in range(CA)]
        wq_sb = [xin.tile([128, DQ], BF16, tag=f"wq{a}", name=f"wqs{a}") for a in range(CA)]
        wk_sb = [xin.tile([128, DQ], BF16, tag=f"wk{a}", name=f"wks{a}") for a in range(CA)]
        wv_sb = [xin.tile([128, DQ], BF16, tag=f"wv{a}", name=f"wvs{a}") for a in range(CA)]

        # PSUM budget (8 banks): ST s_ps 2x[128,1024] = 4 banks,
        # u accumulators 2x[65,512] = 2 banks (halves run sequentially so
        # only one half's pair is live), shared filler pool 2x[128,512]
        # = 2 banks for V / QK projection / out-projection partials.
        stp = ep(tc.tile_pool(name="stp", bufs=2, space="PSUM"))
        u_ps = ep(tc.tile_pool(name="u_ps", bufs=2, space="PSUM"))
        fill_ps = ep(tc.tile_pool(name="fill_ps", bufs=2, space="PSUM"))
        p_pool = ep(tc.tile_pool(name="p_pool", bufs=12))
        norm = ep(tc.tile_pool(name="norm", bufs=4))
        ocp = ep(tc.tile_pool(name="ocp", bufs=2))

        # ---- input DMA, ordered by first use ----
        # Each transfer is worked by a single SDMA engine (~20 GB/s), with
        # ~16 in flight; completion time ~ transfer size. Quarter-split x so
        # the first-needed pieces land early instead of everything at ~27us.
        nc.sync.dma_start(out=tri, in_=trid[:, :])
        for a in range(CA):
            sl = slice(a * 128, (a + 1) * 128)
            nc.sync.dma_start(out=xts[a][:, 0:512], in_=xT[sl, 0:512])
            nc.sync.dma_start(out=wq_sb[a], in_=wq[sl, :])
            nc.sync.dma_start(out=wk_sb[a], in_=wk[sl, :])
        for a in range(CA):
            sl = slice(a * 128, (a + 1) * 128)
            nc.sync.dma_start(out=xts[a][:, 512:1024], in_=xT[sl, 512:1024])
        for a in range(CA):
            sl = slice(a * 128, (a + 1) * 128)
            nc.sync.dma_start(out=wv_sb[a], in_=wv[sl, :])
        for q in (2, 3):
            for a in range(CA):
                sl = slice(a * 128, (a + 1) * 128)
                nc.sync.dma_start(
                    out=xts[a][:, q * 512 : (q + 1) * 512], in_=xT[sl, q * 512 : (q + 1) * 512]
                )
        for j in range(4):
            nc.sync.dma_start(out=wo_sb[j], in_=wo[j * 128 : (j + 1) * 128, :])

        # ---- filler units (each ~0.9-1.7us of PE work) ----
        def emit_v(tk):
            ps = fill_ps.tile([128, 512], F32, tag="fps", name="vps")
            for a in range(CA):
                nc.tensor.matmul(
                    ps,
                    xts[a][:, tk * 128 : (tk + 1) * 128],
                    wv_sb[a],
                    start=(a == 0),
                    stop=(a == CA - 1),
                )
            v_view = vts[tk].rearrange("p (h e) -> p h e", e=65)
            nc.vector.tensor_copy(
                v_view[:, :, 0:64], ps.rearrange("p (h e) -> p h e", e=64)
            )
            nc.vector.memset(v_view[:, :, 64:65], 1.0)

        def emit_qkt_unit(m, wsb, dst, tbp, hb):
            ps = fill_ps.tile([128, 512], F32, tag="fps", name="qkps")
            lo = tbp * 1024 + hb * 512
            for a in range(CA):
                nc.tensor.matmul(
                    ps,
                    wsb[a][:, m * 128 : (m + 1) * 128],
                    xts[a][:, lo : lo + 512],
                    start=(a == 0),
                    stop=(a == CA - 1),
                )
            nc.vector.tensor_copy(dst[m][:, lo : lo + 512], ps)

        def emit_outproj_unit(tq, nb):
            ps = fill_ps.tile([128, 512], F32, tag="fps", name="opps")
            for j in range(4):
                nc.tensor.matmul(
                    ps,
                    hds[j][:, tq * 128 : (tq + 1) * 128],
                    wo_sb[j][:, nb * 512 : (nb + 1) * 512],
                    start=(j == 0),
                    stop=(j == 3),
                )
            ob = ocp.tile([128, 512], F32, tag="ob", name="ob")
            nc.vector.tensor_copy(ob, ps)
            nc.sync.dma_start(
                out=y[tq * 128 : (tq + 1) * 128, nb * 512 : (nb + 1) * 512],
                in_=ob,
            )

        # ---- attention ----
        # Narrow causal-wedge k-chunks are packed in pairs into one PSUM
        # tile / one exp: a "group" is a tuple of ik chunks whose trimmed
        # score regions fit in 1024 columns together. Member regions are
        # concatenated (member ik's q range [qlo+c0, 1024*(qg+1)) maps to
        # p_t columns [base, base+width)). Pairings keep PV accumulation
        # stop order legal: the stop chunk (4*qb+3) stays last per qb.
        def groups_for(qg):
            if qg == 0:
                return [(0,), (1,), (2,), (3,), (4, 6), (5, 7)]
            return [(i,) for i in range(12)] + [(12, 14), (13, 15)]

        def emit_st_exp(m, qg, grp, half):
            # Every matmul output must stay inside one 512-f32 PSUM bank:
            # singles keep the natural [c0:1024] layout (bank-aligned qb
            # splits); pair members sit at bases (0, 512), with the hole
            # [w_a:512] memset to 0 before the shared exp.
            qlo = 1024 * qg
            rq = slice(half * 64, half * 64 + 64)
            s_ps = stp.tile([128, 1024], F32, tag="stps", name="sps")
            p_t = p_pool.tile([128, 1024], BF16, tag="p", name="pt")
            bases = {}
            if len(grp) == 1:
                c0 = max(0, 128 * grp[0] - qlo)
                bases[grp[0]] = (c0, c0)
                exp_lo, exp_hi = c0, 1024
            else:
                w_a = 1024 - max(0, 128 * grp[0] - qlo)
                for base, ik in zip((0, 512), grp):
                    bases[ik] = (base, max(0, 128 * ik - qlo))
                if w_a < 512:
                    nc.vector.memset(s_ps[:, w_a:512], 0.0)
                exp_lo, exp_hi = 0, 512 + 1024 - max(0, 128 * grp[1] - qlo)
            for ik in grp:
                base, c0 = bases[ik]
                kc = slice(ik * 128, (ik + 1) * 128)
                for qb in (2 * qg, 2 * qg + 1):
                    if 4 * qb + 3 < ik:
                        continue
                    lo = max(qb * 512, qlo + c0)  # global q start
                    n = (qb + 1) * 512 - lo
                    col = base + (lo - qlo - c0)
                    nc.tensor.matmul(
                        s_ps[:, col : col + n],
                        kts[m][rq, kc],
                        qts[m][rq, lo : lo + n],
                        start=True,
                        stop=True,
                    )
            nc.scalar.activation(
                p_t[:, exp_lo:exp_hi], s_ps[:, exp_lo:exp_hi], EXP, scale=SCALE
            )
            for ik in grp:
                base, c0 = bases[ik]
                if 128 * ik >= qlo:  # diagonal block: multiplicative mask
                    nc.vector.tensor_mul(
                        p_t[:, base : base + 128], p_t[:, base : base + 128], tri
                    )
            return p_t, bases

        def emit_pv_norm(m, qg, ik, half, p_t, bases, us):
            qlo = 1024 * qg
            base, c0 = bases[ik]
            h = 2 * m + half
            rq = slice(half * 64, half * 64 + 64)
            v_lhsT = vts[ik][:, h * 65 : h * 65 + 65]
            for qb in (2 * qg, 2 * qg + 1):
                if 4 * qb + 3 < ik:
                    continue
                a0 = qb * 512 - qlo
                off = max(0, c0 - a0)  # clip masked cols
                col = base + a0 + off - c0
                nc.tensor.matmul(
                    us[qb][:, off:512],
                    v_lhsT,
                    p_t[:, col : col + 512 - off],
                    start=(ik == 0),
                    stop=(ik == 4 * qb + 3),
                    skip_group_check=True,
                )
                if ik == 4 * qb + 3:
                    # u complete: normalize into hd tiles
                    u = us[qb]
                    rec_in = norm.tile([1, 512], F32, tag="ri", name="ri")
                    nc.vector.tensor_copy(rec_in, u[64:65, :])
                    rec = norm.tile([1, 512], F32, tag="rc", name="rc")
                    nc.vector.reciprocal_approx_fast(out=rec, in_=rec_in)
                    bc = norm.tile([64, 512], F32, tag="bc", name="bc")
                    nc.gpsimd.partition_broadcast(bc, rec)
                    nc.vector.tensor_mul(
                        hds[m][rq, qb * 512 : (qb + 1) * 512], u[0:64, :], bc
                    )

        def attn(qg, m, fillers, front=0, late_fillers=()):
            """ST/exp -> PV pipeline over ik for each half; filler units are
            injected between steps. Tile treats emission order as program
            order, so a consumer emitted before its producer reads stale
            data: the first `front` fillers are paced one-per-step (for
            units consumed by THIS window with a small step margin, e.g. V
            feeding PV), the rest uniformly. `late_fillers` are emitted
            right after half 1's ik==11 PV (for out-proj units gated on
            this window's second-to-last normalize)."""
            grps = groups_for(qg)
            nsteps = 2 * (len(grps) + LAG)
            step = 0
            fill_done = 0
            for half in range(2):
                staged = {}
                us = {}
                for t in range(len(grps) + LAG):
                    if t < len(grps):
                        staged[grps[t]] = emit_st_exp(m, qg, grps[t], half)
                    if t == LAG:
                        for qb in (2 * qg, 2 * qg + 1):
                            us[qb] = u_ps.tile([65, 512], F32, tag="u", name=f"u{qb}")
                    if t >= LAG:
                        grp = grps[t - LAG]
                        p_t, bases = staged.pop(grp)
                        for ik in grp:
                            emit_pv_norm(m, qg, ik, half, p_t, bases, us)
                            if half == 1 and ik == 11 and late_fillers:
                                for u in late_fillers:
                                    u()
                                late_fillers = ()
                    step += 1
                    if fill_done < front:
                        want = min(front, step)
                    else:
                        rem = len(fillers) - front
                        want = front + (step * rem) // nsteps
                    while fill_done < min(want, len(fillers)):
                        fillers[fill_done]()
                        fill_done += 1
            while fill_done < len(fillers):
                fillers[fill_done]()
                fill_done += 1

        def qkt_units(m, tbp):
            out = []
            for wsb, dst in ((wq_sb, qts), (wk_sb, kts)):
                for hb in range(2):
                    out.append(
                        lambda m=m, wsb=wsb, dst=dst, tbp=tbp, hb=hb: emit_qkt_unit(
                            m, wsb, dst, tbp, hb
                        )
                    )
            return out

        def op_units(tqs):
            return [
                lambda tq=tq, nb=nb: emit_outproj_unit(tq, nb)
                for tq in tqs
                for nb in range(2)
            ]

        # ---- schedule ----
        # Pre-phase: QK for pair 0 and the first half of V, emitted
        # serially (their consumers start in the very first window).
        for u in qkt_units(0, 0):
            u()
        for tk in range(8):
            emit_v(tk)

        # qg1 runs pairs in REVERSE order so the out-projection of qg0
        # (available once qg0 completes) can fill the late windows; the
        # last window's shortfall rolls into the dense out-proj tail.
        # V(8..15) is front-loaded into (1,3) (its PV consumes tk=ik with
        # a comfortable emission margin: V(15) by step 8, PV(ik15) at
        # step 18).
        windows = [
            (0, 0, qkt_units(1, 0), 0, ()),
            (0, 1, qkt_units(2, 0), 0, ()),
            (0, 2, qkt_units(3, 0), 0, ()),
            (0, 3, qkt_units(3, 1), 0, ()),
            (
                1,
                3,
                [lambda tk=tk: emit_v(tk) for tk in range(8, 16)]
                + qkt_units(2, 1)
                + op_units(range(0, 2)),
                8,
                (),
            ),
            (1, 2, qkt_units(1, 1) + op_units(range(2, 4)), 0, ()),
            (1, 1, qkt_units(0, 1) + op_units(range(4, 6)), 0, ()),
            (1, 0, op_units(range(6, 8)), 0, op_units(range(8, 12))),
        ]
        for qg, m, fillers, front, late in windows:
            attn(qg, m, fillers, front=front, late_fillers=late)
        for u in op_units(range(12, 16)):
            u()


def _in_maps(x, w_qkv, w_out):
    maps = []
    for c in range(NCORES):
        b, g = c // 2, c % 2
        h0 = g * DQ
        maps.append(
            {
                "xT": np.ascontiguousarray(x[b].T).astype(NP_BF16),
                "wq": w_qkv[:, h0 : h0 + DQ].astype(NP_BF16),
                "wk": w_qkv[:, C + h0 : C + h0 + DQ].astype(NP_BF16),
                "wv": w_qkv[:, 2 * C + h0 : 2 * C + h0 + DQ].astype(NP_BF16),
                "wo": np.ascontiguousarray(w_out[h0 : h0 + DQ, :]).astype(NP_BF16),
                "tri": np.triu(np.ones((128, 128), dtype=np.float32)).astype(NP_BF16),
            }
        )
    return maps


def get_bass():
    global _cached
    if _cached is None:
        _cached = _build()
    return _cached


def run(x, w_qkv, w_out, b_out, **spmd_kwargs):
    nc = get_bass()
    maps = _in_maps(x, w_qkv, w_out)
    try:
        res = run_bass_kernel_spmd(
            nc, maps, core_ids=list(range(NCORES)), **spmd_kwargs
        )
    except Exception:
        # transient device errors (NRT unrecoverable) — one retry
        res = run_bass_kernel_spmd(
            nc, maps, core_ids=list(range(NCORES)), **spmd_kwargs
        )
    out = np.empty((B, T, C), dtype=np.float32)
    for b in range(B):
        out[b] = res.results[2 * b]["y"] + res.results[2 * b + 1]["y"]
    out += b_out.astype(np.float32)
    return out, res


def kernel(x, w_qkv, w_out, b_out):
    x = np.asarray(x)
    w_qkv = np.asarray(w_qkv)
    w_out = np.asarray(w_out)
    b_out = np.asarray(b_out)
    out, _ = run(x, w_qkv, w_out, b_out)
    return out


if __name__ == "__main__":
    import reference

    inputs = {k: np.asarray(v) for k, v in reference.setup_inputs().items()}
    out = kernel(**inputs)
    print(out.shape, out.dtype)


# revision 15
# speedup vs baseline: 1.2022x; 1.1535x over previous
"""Multi-head causal self-attention forward on 8 Trainium2 NeuronCores.

Problem: x[4,2048,1024] @ w_qkv[1024,3072] -> causal MHA (16 heads, d=64)
         -> @ w_out[1024,1024] + b_out.

Sharding: core c handles batch b = c//2 and head-group g = c%2 (8 heads).
Each core computes a partial output  attn_out_heads(g) @ w_out[rows(g)]
for its batch; host sums the two partials per batch (row-parallel out
projection) and adds b_out.

Per-core kernel (bf16 matmul inputs, fp32 PSUM accumulate), organized to
keep the PE array continuously fed (TRN2 HAM drops the PE to half clock
after idle gaps):
  - q-group-outer / head-pair-inner loop. Attention for (qg, m) runs
    ST -> exp -> PV with sequentially processed head halves; all
    projection work (V, Q/K for later pairs, out-projection of finished
    q-groups) is chopped into ~1.7us "filler" units and injected
    between attention steps so the PE never idles while the ScalarE
    exp stream catches up.
  - attention in [k,q] orientation: S^T = kT.T @ qT per (k-chunk 128,
    q-group 1024); exp on ScalarE with scale=1/8 folded in; causal via
    block skip + triangular-mask multiply on the diagonal block;
    PV: u[65,512] += [v|1].T @ P accumulated over k-chunks ([v|1]
    carries a ones-column so u row 64 is the softmax denominator).
  - normalize with reciprocal_approx_fast + gpsimd partition_broadcast
    + DVE multiply -> hd tiles [hd, q] bf16.
  - out = hd.T @ w_out_shard accumulated over 4 hd chunks -> y f32,
    PSUM->SBUF copy on DVE (not ScalarE), DMA per [128,512] block.
  - input DMA split by token halves and ordered by first use (x+wq/wk
    first) so the first matmuls start ~5us in instead of waiting for
    the whole 8MB input set.
"""

import sys

sys.path.insert(0, "/opt/trn_rl_repo")

import numpy as np
import ml_dtypes

import concourse.bass as bass
import concourse.tile as tile
from concourse import bacc, mybir
from concourse.bass_utils import run_bass_kernel_spmd

BF16 = mybir.dt.bfloat16
F32 = mybir.dt.float32
NP_BF16 = ml_dtypes.bfloat16
EXP = mybir.ActivationFunctionType.Exp

B, T, C = 4, 2048, 1024
NCORES = 8
HC = 8  # heads per core
D = 64
DQ = HC * D  # 512
CA = C // 128  # 8 contraction chunks
NT128 = T // 128  # 16
SCALE = 1.0 / 8.0
LAG = 2  # ST/exp stream runs this many ik steps ahead of PV

_cached = None


def _build():
    nc = bacc.Bacc("TRN2", target_bir_lowering=False, debug=False, num_devices=NCORES)

    xT = nc.dram_tensor("xT", [C, T], BF16, kind="ExternalInput")
    wq = nc.dram_tensor("wq", [C, DQ], BF16, kind="ExternalInput")
    wk = nc.dram_tensor("wk", [C, DQ], BF16, kind="ExternalInput")
    wv = nc.dram_tensor("wv", [C, DQ], BF16, kind="ExternalInput")
    wo = nc.dram_tensor("wo", [DQ, C], BF16, kind="ExternalInput")
    trid = nc.dram_tensor("tri", [128, 128], BF16, kind="ExternalInput")
    # bf16 partial outputs: host upcasts and sums the two head-group
    # partials in f32; halves the output DMA so the kernel tail drains
    # ~2x faster. Rounding the partials costs ~0.1-0.3% rel error
    # against a 2e-2 budget.
    y = nc.dram_tensor("y", [T, C], BF16, kind="ExternalOutput")

    with tile.TileContext(nc) as tc:
        _emit(tc, nc, xT, wq, wk, wv, wo, trid, y)
    nc.compile()
    return nc


def _emit(tc, nc, xT, wq, wk, wv, wo, trid, y):
    from contextlib import ExitStack

    with ExitStack() as ctx:
        ep = ctx.enter_context

        persist = ep(tc.tile_pool(name="persist", bufs=1))
        qts = [persist.tile([128, T], BF16, tag=f"qt{m}", name=f"qt{m}") for m in range(4)]
        kts = [persist.tile([128, T], BF16, tag=f"kt{m}", name=f"kt{m}") for m in range(4)]
        vts = [persist.tile([128, HC * 65], BF16, tag=f"v{i}", name=f"v{i}") for i in range(NT128)]
        hds = [persist.tile([128, T], BF16, tag=f"hd{j}", name=f"hd{j}") for j in range(4)]
        wo_sb = [persist.tile([128, C], BF16, tag=f"wo{j}", name=f"wo{j}") for j in range(4)]
        tri = persist.tile([128, 128], BF16, tag="tri", name="tri")

        xin = ep(tc.tile_pool(name="xin", bufs=1))
        xts = [xin.tile([128, T], BF16, tag=f"x{a}", name=f"x{a}") for a in range(CA)]
        wq_sb = [xin.tile([128, DQ], BF16, tag=f"wq{a}", name=f"wqs{a}") for a in range(CA)]
        wk_sb = [xin.tile([128, DQ], BF16, tag=f"wk{a}", name=f"wks{a}") for a in range(CA)]
        wv_sb = [xin.tile([128, DQ], BF16, tag=f"wv{a}", name=f"wvs{a}") for a in range(CA)]

        # PSUM budget (8 banks): ST s_ps 2x[128,1024] = 4 banks,
        # u accumulators 2x[65,512] = 2 banks (halves run sequentially so
        # only one half's pair is live), shared filler pool 2x[128,512]
        # = 2 banks for V / QK projection / out-projection partials.
        stp = ep(tc.tile_pool(name="stp", bufs=2, space="PSUM"))
        u_ps = ep(tc.tile_pool(name="u_ps", bufs=2, space="PSUM"))
        fill_ps = ep(tc.tile_pool(name="fill_ps", bufs=2, space="PSUM"))
        p_pool = ep(tc.tile_pool(name="p_pool", bufs=12))
        norm = ep(tc.tile_pool(name="norm", bufs=4))
        ocp = ep(tc.tile_pool(name="ocp", bufs=2))

        # ---- input DMA, ordered by first use ----
        # Each transfer is worked by a single SDMA engine (~20 GB/s), with
        # ~16 in flight; completion time ~ transfer size. Quarter-split x so
        # the first-needed pieces land early instead of everything at ~27us.
        nc.sync.dma_start(out=tri, in_=trid[:, :])
        for a in range(CA):
            sl = slice(a * 128, (a + 1) * 128)
            nc.sync.dma_start(out=xts[a][:, 0:512], in_=xT[sl, 0:512])
            nc.sync.dma_start(out=wq_sb[a], in_=wq[sl, :])
            nc.sync.dma_start(out=wk_sb[a], in_=wk[sl, :])
        for a in range(CA):
            sl = slice(a * 128, (a + 1) * 128)
            nc.sync.dma_start(out=xts[a][:, 512:1024], in_=xT[sl, 512:1024])
        for a in range(CA):
            sl = slice(a * 128, (a + 1) * 128)
            nc.sync.dma_start(out=wv_sb[a], in_=wv[sl, :])
        for q in (2, 3):
            for a in range(CA):
                sl = slice(a * 128, (a + 1) * 128)
                nc.sync.dma_start(
                    out=xts[a][:, q * 512 : (q + 1) * 512], in_=xT[sl, q * 512 : (q + 1) * 512]
                )
        for j in range(4):
            nc.sync.dma_start(out=wo_sb[j], in_=wo[j * 128 : (j + 1) * 128, :])

        # ---- filler units (each ~0.9-1.7us of PE work) ----
        def emit_v(tk):
            ps = fill_ps.tile([128, 512], F32, tag="fps", name="vps")
            for a in range(CA):
                nc.tensor.matmul(
                    ps,
                    xts[a][:, tk * 128 : (tk + 1) * 128],
                    wv_sb[a],
                    start=(a == 0),
                    stop=(a == CA - 1),
                )
            v_view = vts[tk].rearrange("p (h e) -> p h e", e=65)
            nc.vector.tensor_copy(
                v_view[:, :, 0:64], ps.rearrange("p (h e) -> p h e", e=64)
            )
            nc.vector.memset(v_view[:, :, 64:65], 1.0)

        def emit_qkt_unit(m, wsb, dst, tbp, hb):
            ps = fill_ps.tile([128, 512], F32, tag="fps", name="qkps")
            lo = tbp * 1024 + hb * 512
            for a in range(CA):
                nc.tensor.matmul(
                    ps,
                    wsb[a][:, m * 128 : (m + 1) * 128],
                    xts[a][:, lo : lo + 512],
                    start=(a == 0),
                    stop=(a == CA - 1),
                )
            nc.vector.tensor_copy(dst[m][:, lo : lo + 512], ps)

        def emit_outproj_unit(tq, nb):
            ps = fill_ps.tile([128, 512], F32, tag="fps", name="opps")
            for j in range(4):
                nc.tensor.matmul(
                    ps,
                    hds[j][:, tq * 128 : (tq + 1) * 128],
                    wo_sb[j][:, nb * 512 : (nb + 1) * 512],
                    start=(j == 0),
                    stop=(j == 3),
                )
            ob = ocp.tile([128, 512], BF16, tag="ob", name="ob")
            nc.vector.tensor_copy(ob, ps)
            # split the store so two SDMA engines carry it (one engine
            # moves ~20 GB/s; the final stores gate the kernel drain)
            for hh in range(2):
                nc.sync.dma_start(
                    out=y[
                        tq * 128 : (tq + 1) * 128,
                        nb * 512 + hh * 256 : nb * 512 + (hh + 1) * 256,
                    ],
                    in_=ob[:, hh * 256 : (hh + 1) * 256],
                )

        # ---- attention ----
        # Narrow causal-wedge k-chunks are packed in pairs into one PSUM
        # tile / one exp: a "group" is a tuple of ik chunks whose trimmed
        # score regions fit in 1024 columns together. Member regions are
        # concatenated (member ik's q range [qlo+c0, 1024*(qg+1)) maps to
        # p_t columns [base, base+width)). Pairings keep PV accumulation
        # stop order legal: the stop chunk (4*qb+3) stays last per qb.
        def groups_for(qg):
            if qg == 0:
                return [(0,), (1,), (2,), (3,), (4, 6), (5, 7)]
            return [(i,) for i in range(12)] + [(12, 14), (13, 15)]

        def emit_st_exp(m, qg, grp, half):
            # Every matmul output must stay inside one 512-f32 PSUM bank:
            # singles keep the natural [c0:1024] layout (bank-aligned qb
            # splits); pair members sit at bases (0, 512), with the hole
            # [w_a:512] memset to 0 before the shared exp.
            qlo = 1024 * qg
            rq = slice(half * 64, half * 64 + 64)
            s_ps = stp.tile([128, 1024], F32, tag="stps", name="sps")
            p_t = p_pool.tile([128, 1024], BF16, tag="p", name="pt")
            bases = {}
            if len(grp) == 1:
                c0 = max(0, 128 * grp[0] - qlo)
                bases[grp[0]] = (c0, c0)
                exp_lo, exp_hi = c0, 1024
            else:
                w_a = 1024 - max(0, 128 * grp[0] - qlo)
                for base, ik in zip((0, 512), grp):
                    bases[ik] = (base, max(0, 128 * ik - qlo))
                if w_a < 512:
                    nc.vector.memset(s_ps[:, w_a:512], 0.0)
                exp_lo, exp_hi = 0, 512 + 1024 - max(0, 128 * grp[1] - qlo)
            for ik in grp:
                base, c0 = bases[ik]
                kc = slice(ik * 128, (ik + 1) * 128)
                for qb in (2 * qg, 2 * qg + 1):
                    if 4 * qb + 3 < ik:
                        continue
                    lo = max(qb * 512, qlo + c0)  # global q start
                    n = (qb + 1) * 512 - lo
                    col = base + (lo - qlo - c0)
                    nc.tensor.matmul(
                        s_ps[:, col : col + n],
                        kts[m][rq, kc],
                        qts[m][rq, lo : lo + n],
                        start=True,
                        stop=True,
                    )
            nc.scalar.activation(
                p_t[:, exp_lo:exp_hi], s_ps[:, exp_lo:exp_hi], EXP, scale=SCALE
            )
            for ik in grp:
                base, c0 = bases[ik]
                if 128 * ik >= qlo:  # diagonal block: multiplicative mask
                    nc.vector.tensor_mul(
                        p_t[:, base : base + 128], p_t[:, base : base + 128], tri
                    )
            return p_t, bases

        def emit_pv_norm(m, qg, ik, half, p_t, bases, us):
            qlo = 1024 * qg
            base, c0 = bases[ik]
            h = 2 * m + half
            rq = slice(half * 64, half * 64 + 64)
            v_lhsT = vts[ik][:, h * 65 : h * 65 + 65]
            for qb in (2 * qg, 2 * qg + 1):
                if 4 * qb + 3 < ik:
                    continue
                a0 = qb * 512 - qlo
                off = max(0, c0 - a0)  # clip masked cols
                col = base + a0 + off - c0
                nc.tensor.matmul(
                    us[qb][:, off:512],
                    v_lhsT,
                    p_t[:, col : col + 512 - off],
                    start=(ik == 0),
                    stop=(ik == 4 * qb + 3),
                    skip_group_check=True,
                )
                if ik == 4 * qb + 3:
                    # u complete: normalize into hd tiles
                    u = us[qb]
                    rec_in = norm.tile([1, 512], F32, tag="ri", name="ri")
                    nc.vector.tensor_copy(rec_in, u[64:65, :])
                    rec = norm.tile([1, 512], F32, tag="rc", name="rc")
                    nc.vector.reciprocal_approx_fast(out=rec, in_=rec_in)
                    bc = norm.tile([64, 512], F32, tag="bc", name="bc")
                    nc.gpsimd.partition_broadcast(bc, rec)
                    nc.vector.tensor_mul(
                        hds[m][rq, qb * 512 : (qb + 1) * 512], u[0:64, :], bc
                    )

        def attn(qg, m, fillers, front=0, late_fillers=()):
            """ST/exp -> PV pipeline over ik for each half; filler units are
            injected between steps. Tile treats emission order as program
            order, so a consumer emitted before its producer reads stale
            data: the first `front` fillers are paced one-per-step (for
            units consumed by THIS window with a small step margin, e.g. V
            feeding PV), the rest uniformly. `late_fillers` are emitted
            right after half 1's ik==11 PV (for out-proj units gated on
            this window's second-to-last normalize)."""
            grps = groups_for(qg)
            nsteps = 2 * (len(grps) + LAG)
            step = 0
            fill_done = 0
            for half in range(2):
                staged = {}
                us = {}
                for t in range(len(grps) + LAG):
                    if t < len(grps):
                        staged[grps[t]] = emit_st_exp(m, qg, grps[t], half)
                    if t == LAG:
                        for qb in (2 * qg, 2 * qg + 1):
                            us[qb] = u_ps.tile([65, 512], F32, tag="u", name=f"u{qb}")
                    if t >= LAG:
                        grp = grps[t - LAG]
                        p_t, bases = staged.pop(grp)
                        for ik in grp:
                            emit_pv_norm(m, qg, ik, half, p_t, bases, us)
                            if half == 1 and ik == 11 and late_fillers:
                                for u in late_fillers:
                                    u()
                                late_fillers = ()
                    step += 1
                    if fill_done < front:
                        want = min(front, step)
                    else:
                        rem = len(fillers) - front
                        want = front + (step * rem) // nsteps
                    while fill_done < min(want, len(fillers)):
                        fillers[fill_done]()
                        fill_done += 1
            while fill_done < len(fillers):
                fillers[fill_done]()
                fill_done += 1

        def qkt_units(m, tbp):
            out = []
            for wsb, dst in ((wq_sb, qts), (wk_sb, kts)):
                for hb in range(2):
                    out.append(
                        lambda m=m, wsb=wsb, dst=dst, tbp=tbp, hb=hb: emit_qkt_unit(
                            m, wsb, dst, tbp, hb
                        )
                    )
            return out

        def op_units(tqs):
            return [
                lambda tq=tq, nb=nb: emit_outproj_unit(tq, nb)
                for tq in tqs
                for nb in range(2)
            ]

        # ---- schedule ----
        # Pre-phase: QK for pair 0 and the first half of V, emitted
        # serially (their consumers start in the very first window).
        for u in qkt_units(0, 0):
            u()
        for tk in range(8):
            emit_v(tk)

        # qg1 runs pairs in REVERSE order so the out-projection of qg0
        # (available once qg0 completes) can fill the late windows; the
        # last window's shortfall rolls into the dense out-proj tail.
        # V(8..15) is front-loaded into (1,3) (its PV consumes tk=ik with
        # a comfortable emission margin: V(15) by step 8, PV(ik15) at
        # step 18).
        windows = [
            (0, 0, qkt_units(1, 0), 0, ()),
            (0, 1, qkt_units(2, 0), 0, ()),
            (0, 2, qkt_units(3, 0), 0, ()),
            (0, 3, qkt_units(3, 1), 0, ()),
            (
                1,
                3,
                [lambda tk=tk: emit_v(tk) for tk in range(8, 16)]
                + qkt_units(2, 1)
                + op_units(range(0, 2)),
                8,
                (),
            ),
            (1, 2, qkt_units(1, 1) + op_units(range(2, 4)), 0, ()),
            (1, 1, qkt_units(0, 1) + op_units(range(4, 6)), 0, ()),
            (1, 0, op_units(range(6, 8)), 0, op_units(range(8, 12))),
        ]
        for qg, m, fillers, front, late in windows:
            attn(qg, m, fillers, front=front, late_fillers=late)
        for u in op_units(range(12, 16)):
            u()


def _in_maps(x, w_qkv, w_out):
    maps = []
    for c in range(NCORES):
        b, g = c // 2, c % 2
        h0 = g * DQ
        maps.append(
            {
                "xT": np.ascontiguousarray(x[b].T).astype(NP_BF16),
                "wq": w_qkv[:, h0 : h0 + DQ].astype(NP_BF16),
                "wk": w_qkv[:, C + h0 : C + h0 + DQ].astype(NP_BF16),
                "wv": w_qkv[:, 2 * C + h0 : 2 * C + h0 + DQ].astype(NP_BF16),
                "wo": np.ascontiguousarray(w_out[h0 : h0 + DQ, :]).astype(NP_BF16),
                "tri": np.triu(np.ones((128, 128), dtype=np.float32)).astype(NP_BF16),
            }
        )
    return maps


def get_bass():
    global _cached
    if _cached is None:
        _cached = _build()
    return _cached


def run(x, w_qkv, w_out, b_out, **spmd_kwargs):
    nc = get_bass()
    maps = _in_maps(x, w_qkv, w_out)
    try:
        res = run_bass_kernel_spmd(
            nc, maps, core_ids=list(range(NCORES)), **spmd_kwargs
        )
    except Exception:
        # transient device errors (NRT unrecoverable) — one retry
        res = run_bass_kernel_spmd(
            nc, maps, core_ids=list(range(NCORES)), **spmd_kwargs
        )
    out = np.empty((B, T, C), dtype=np.float32)
    for b in range(B):
        out[b] = res.results[2 * b]["y"].astype(np.float32) + res.results[
            2 * b + 1
        ]["y"].astype(np.float32)
    out += b_out.astype(np.float32)
    return out, res


def kernel(x, w_qkv, w_out, b_out):
    x = np.asarray(x)
    w_qkv = np.asarray(w_qkv)
    w_out = np.asarray(w_out)
    b_out = np.asarray(b_out)
    out, _ = run(x, w_qkv, w_out, b_out)
    return out


if __name__ == "__main__":
    import reference

    inputs = {k: np.asarray(v) for k, v in reference.setup_inputs().items()}
    out = kernel(**inputs)
    print(out.shape, out.dtype)
